# revision 1
# baseline (speedup 1.0000x reference)
"""Trainium2 Bass kernel for nn_ContinuousSoftmax.

Computes r[n,j] = N(Mu_n - mu_j; 0, Sigma_n + S_j) for N=131072 rows and
M=100 Gaussian basis functions, where Sigma_n/Mu_n derive from natural
parameters theta[n, :6].

Math: with s00,s01,s11 the entries of Sigma_n, Mu the mean, and basis
(a,b,g) = (S00, S11, sym-offdiag) per basis j:
    C   = Sigma + S_j         (2x2 symmetric)
    det = c00*c11 - c01^2
    quad = (c11*d0^2 + c00*d1^2 - 2*c01*d0*d1) / det,  d = Mu - mu_j
    r = exp(-0.5*quad) / (2*pi*sqrt(det))

Both num = quad*det and det are bilinear in (per-n monomials) x (per-j
monomials) with rank 15, so PE matmuls produce num and det for all bases.
Two 128-row blocks are packed per matmul (K=30, block-diagonal B table,
512-wide moving operand so fp32r streams at 1 cycle/row), and four such
pair-matmuls run concurrently in separate 32-row PE groups. The remaining
elementwise work is r = Exp(-0.5*(num/det + Ln(4*pi^2*det))) — Ln and Exp
share one ACT table so no table reloads occur.

Sharding: pure data-parallel over 8 NeuronCores along N (16384 rows each).
"""

import numpy as np

N_CORES = 8
N_TOTAL = 131072
N_LOCAL = N_TOTAL // N_CORES  # 16384
M = 100
P = 128                # SBUF partitions
K = 15                 # bilinear rank (A columns / B rows)
QB = 4                 # blocks packed per quad-matmul (contraction K*QB=60)
KQ = K * QB            # 60
QUADS = 2              # quads per chunk
CB = QB * QUADS        # blocks (of 128 rows) per chunk = 8
MM_N = QB * M          # moving free dim of a quad-matmul = 400
TWO_PI = 6.283185307179586
FOUR_PI2 = 39.47841760435743  # (2*pi)^2

# "f32r" (fast, reduced-precision PE mode) or "f32" (exact, 4x slower PE)
MM_DTYPE = "f32r"
# quad via one tensor_tensor divide (requires one operand in SBUF;
# both live in PSUM so this must stay False), else reciprocal + multiply
USE_DIVIDE = False

_CACHE = {}


def _build_b_table(basis_mu, basis_sigma):
    """Host-side [2*KQ, MM_N] coefficient table from the small basis tables.

    Rows 0:KQ are the num-table, rows KQ:2*KQ the det-table. Each is
    block-diagonal over the QB blocks of a quad: row K*d+k, cols M*d:M*(d+1)
    hold the rank-15 coefficient row k (num resp. det) for quad-local block
    d, so one [KQ,128] x [KQ,400] matmul evaluates 4 blocks at once."""
    mu0 = basis_mu[:, 0].astype(np.float64)
    mu1 = basis_mu[:, 1].astype(np.float64)
    a = basis_sigma[:, 0, 0].astype(np.float64)
    b = basis_sigma[:, 1, 1].astype(np.float64)
    g = 0.5 * (basis_sigma[:, 0, 1] + basis_sigma[:, 1, 0]).astype(np.float64)

    Bn = np.zeros((K, M), dtype=np.float64)
    Bd = np.zeros((K, M), dtype=np.float64)
    Bn[0] = 1.0
    Bn[1] = -2.0 * mu0
    Bn[2] = mu0 * mu0
    Bd[2] = a
    Bn[3] = b
    Bn[4] = -2.0 * b * mu0 + 2.0 * g * mu1
    Bn[5] = -2.0 * mu1
    Bn[6] = mu1 * mu1
    Bd[6] = b
    Bn[7] = a
    Bn[8] = -2.0 * a * mu1 + 2.0 * g * mu0
    Bn[9] = 2.0 * mu1
    Bn[10] = 2.0 * mu0
    Bn[11] = -2.0 * mu0 * mu1
    Bd[11] = -2.0 * g
    Bn[12] = b * mu0 * mu0 + a * mu1 * mu1 - 2.0 * g * mu0 * mu1
    Bd[12] = a * b - g * g
    Bd[13] = 1.0
    Bn[14] = -2.0 * g

    btab = np.zeros((2 * KQ, MM_N), dtype=np.float32)
    for d in range(QB):
        btab[K * d : K * (d + 1), M * d : M * (d + 1)] = Bn.astype(np.float32)
        btab[KQ + K * d : KQ + K * (d + 1), M * d : M * (d + 1)] = Bd.astype(
            np.float32
        )
    return btab


def _emit_planes(nc, pl, theta_t, a_tile, mybir):
    """Per-n prep: from theta planes compute Sigma, Mu and the 15 A columns,
    writing each into a_tile[:, :, k] (stride-K slots).

    A columns:
      0: s11*Mu0^2 + s00*Mu1^2 - 2*s01*Mu0*Mu1   8: Mu1
      1: s11*Mu0                                  9: s01*Mu0
      2: s11                                     10: s01*Mu1
      3: Mu0^2                                   11: s01
      4: Mu0                                     12: 1
      5: s00*Mu1                                 13: s00*s11 - s01^2
      6: s00                                     14: Mu0*Mu1
      7: Mu1^2
    """
    f32 = mybir.dt.float32
    Alu = mybir.AluOpType
    f_tot = theta_t.shape[1]

    def t(tag):
        return pl.tile([P, f_tot], f32, tag=tag, name=tag)

    th = [theta_t[:, :, c] for c in range(6)]

    # detP/4 = t2*t5 - t3*t4 ;  q = 4/detP
    w1 = t("w1")
    nc.gpsimd.tensor_mul(w1, th[2], th[5])
    w2 = t("w2")
    nc.gpsimd.tensor_mul(w2, th[3], th[4])
    dp = t("dp")
    nc.gpsimd.tensor_sub(dp, w1, w2)
    q = t("q")
    nc.vector.reciprocal(q, dp)

    # Sigma entries: s00 = -0.5*t5*q ; s11 = -0.5*t2*q ; s01 = 0.25*(t3+t4)*q
    s00 = a_tile[:, :, 6]
    nc.vector.scalar_tensor_tensor(s00, th[5], -0.5, q, Alu.mult, Alu.mult)
    s11 = a_tile[:, :, 2]
    nc.vector.scalar_tensor_tensor(s11, th[2], -0.5, q, Alu.mult, Alu.mult)
    h34 = t("h34")
    nc.gpsimd.tensor_add(h34, th[3], th[4])
    s01 = a_tile[:, :, 11]
    nc.vector.scalar_tensor_tensor(s01, h34, 0.25, q, Alu.mult, Alu.mult)

    # Mu = Sigma @ eta
    v1 = t("v1")
    nc.gpsimd.tensor_mul(v1, s00, th[0])
    v2 = t("v2")
    nc.gpsimd.tensor_mul(v2, s01, th[1])
    mu0 = a_tile[:, :, 4]
    nc.gpsimd.tensor_add(mu0, v1, v2)
    v3 = t("v3")
    nc.gpsimd.tensor_mul(v3, s01, th[0])
    v4 = t("v4")
    nc.gpsimd.tensor_mul(v4, s11, th[1])
    mu1 = a_tile[:, :, 8]
    nc.gpsimd.tensor_add(mu1, v3, v4)

    # second-order monomials
    m00 = a_tile[:, :, 3]
    nc.gpsimd.tensor_mul(m00, mu0, mu0)
    m11 = a_tile[:, :, 7]
    nc.gpsimd.tensor_mul(m11, mu1, mu1)
    m01 = a_tile[:, :, 14]
    nc.gpsimd.tensor_mul(m01, mu0, mu1)

    # A0 = s11*m00 + s00*m11 - 2*s01*m01
    x1 = t("x1")
    nc.vector.tensor_mul(x1, s11, m00)
    x2 = t("x2")
    nc.vector.tensor_mul(x2, s00, m11)
    x3 = t("x3")
    nc.vector.tensor_mul(x3, s01, m01)
    x12 = t("x12")
    nc.gpsimd.tensor_add(x12, x1, x2)
    nc.vector.scalar_tensor_tensor(
        a_tile[:, :, 0], x3, -2.0, x12, Alu.mult, Alu.add
    )

    # cross monomials
    nc.vector.tensor_mul(a_tile[:, :, 1], s11, mu0)
    nc.vector.tensor_mul(a_tile[:, :, 5], s00, mu1)
    nc.gpsimd.tensor_mul(a_tile[:, :, 9], s01, mu0)
    nc.gpsimd.tensor_mul(a_tile[:, :, 10], s01, mu1)

    # A13 = s00*s11 - s01^2
    y1 = t("y1")
    nc.vector.tensor_mul(y1, s00, s11)
    y2 = t("y2")
    nc.gpsimd.tensor_mul(y2, s01, s01)
    nc.vector.tensor_sub(a_tile[:, :, 13], y1, y2)

    # A12 = 1
    nc.gpsimd.memset(a_tile[:, :, 12], 1.0)


def _build_program(n_local=N_LOCAL):
    """Build + schedule the single-core Bass program (shared by all 8 cores)."""
    import concourse.bass as bass  # noqa: F401
    import concourse.tile as tile
    from concourse import bacc, mybir
    from concourse.masks import make_identity

    f32 = mybir.dt.float32
    f32r = mybir.dt.float32r
    mm_dt = f32r if MM_DTYPE == "f32r" else f32
    Act = mybir.ActivationFunctionType
    Alu = mybir.AluOpType

    f_tot = n_local // P        # number of 128-row blocks
    nchunk = f_tot // CB
    assert f_tot % CB == 0

    nc = bacc.Bacc("TRN2", target_bir_lowering=False, debug=False)

    theta_d = nc.dram_tensor("theta", [n_local, 6], f32, kind="ExternalInput").ap()
    btab_d = nc.dram_tensor("btab", [2 * KQ, MM_N], f32, kind="ExternalInput").ap()
    r_d = nc.dram_tensor("r", [n_local, M], f32, kind="ExternalOutput").ap()

    theta_v = theta_d.rearrange("(p f) c -> p f c", p=P)   # [128, f_tot, 6]
    r_v = r_d.rearrange("(p f) m -> p f m", p=P)           # [128, f_tot, M]

    with tile.TileContext(nc) as tc:
        with (
            tc.tile_pool(name="consts", bufs=1) as consts,
            tc.tile_pool(name="pl", bufs=1) as pl,
            tc.tile_pool(name="atp", bufs=4) as atp,
            tc.tile_pool(name="psum", bufs=2, space="PSUM") as psum,
            tc.tile_pool(name="ew", bufs=3) as ew,
            tc.tile_pool(name="rout", bufs=3) as rout,
        ):
            # ---- constants & inputs ----
            # two tables in separate tiles so both matmul operands sit at
            # partition base 0 (hardware restriction); converted to mm_dt on
            # device because fp32r matmul inputs must be produced as fp32r
            btab_nsb = consts.tile([KQ, MM_N], f32, tag="btabn", name="btab_nsb")
            nc.sync.dma_start(out=btab_nsb, in_=btab_d[0:KQ, :])
            btab_dsb = consts.tile([KQ, MM_N], f32, tag="btabd", name="btab_dsb")
            nc.sync.dma_start(out=btab_dsb, in_=btab_d[KQ : 2 * KQ, :])
            btab_n = consts.tile([KQ, MM_N], mm_dt, tag="btabnr", name="btab_n")
            nc.scalar.copy(out=btab_n, in_=btab_nsb)
            btab_dd = consts.tile([KQ, MM_N], mm_dt, tag="btabdr", name="btab_dd")
            nc.scalar.copy(out=btab_dd, in_=btab_dsb)
            ident = consts.tile([P, P], f32, tag="ident", name="ident")
            make_identity(nc, ident)

            theta_t = consts.tile([P, f_tot, 6], f32, tag="theta", name="theta_t")
            nc.sync.dma_start(out=theta_t, in_=theta_v)

            # ---- per-n prep: A columns ----
            a_tile = consts.tile([P, f_tot, K], f32, tag="a", name="a_tile")
            _emit_planes(nc, pl, theta_t, a_tile, mybir)

            # ---- main loop over chunks of CB blocks (QUADS quads) ----
            for c in range(nchunk):
                # PSUM slot: per quad, one bank for num and one for det
                slot = psum.tile([P, 2 * QUADS, 512], f32, tag="mm", name="mmslot")

                # PE transposes of both quads' A columns [128,60]->[60,128],
                # side by side in bank 0, drained with one PSUM->SBUF copy
                for qd in range(QUADS):
                    q0 = c * CB + qd * QB
                    nc.tensor.transpose(
                        slot[0:KQ, 0, qd * P : (qd + 1) * P],
                        a_tile[:, q0 : q0 + QB, :],
                        ident,
                    )
                at_sb = atp.tile([KQ, QUADS * P], mm_dt, tag="at", name="at_sb")
                nc.scalar.copy(out=at_sb, in_=slot[0:KQ, 0, 0 : QUADS * P])

                # per quad: two matmuls (num and det) for 4 blocks at once
                for qd in range(QUADS):
                    at_mm = at_sb[:, qd * P : (qd + 1) * P]
                    nc.tensor.matmul(
                        slot[:, 2 * qd, 0:MM_N],
                        at_mm,
                        btab_n,
                        start=True,
                        stop=True,
                    )
                    nc.tensor.matmul(
                        slot[:, 2 * qd + 1, 0:MM_N],
                        at_mm,
                        btab_dd,
                        start=True,
                        stop=True,
                    )

                # views over the chunk: [128, QUADS, QB, M]
                num = slot[:, 0::2, 0:MM_N].rearrange("p q (d j) -> p q d j", j=M)
                det = slot[:, 1::2, 0:MM_N].rearrange("p q (d j) -> p q d j", j=M)

                # r = Exp(-0.5*(num/det + Ln(4pi^2 * det)))
                y = ew.tile([P, QUADS, QB, M], f32, tag="y", name="y")
                if USE_DIVIDE:
                    nc.vector.tensor_tensor(out=y, in0=num, in1=det, op=Alu.divide)
                else:
                    recip = ew.tile(
                        [P, QUADS, QB, M], f32, tag="recip", name="recip"
                    )
                    nc.vector.reciprocal(recip, det)
                    nc.vector.tensor_mul(y, num, recip)
                lg = ew.tile([P, QUADS, QB, M], f32, tag="lg", name="lg")
                nc.scalar.activation(lg, det, Act.Ln, scale=float(FOUR_PI2))
                z = ew.tile([P, QUADS, QB, M], f32, tag="z", name="z")
                # balance the add between POOL and DVE (POOL tensor ops run
                # at ~0.42 of line rate; DVE takes every 8th chunk)
                if c % 8 == 0:
                    nc.vector.tensor_add(z, y, lg)
                else:
                    nc.gpsimd.tensor_add(z, y, lg)
                r_t = rout.tile([P, QUADS, QB, M], f32, tag="r", name="r_t")
                nc.scalar.activation(r_t, z, Act.Exp, scale=-0.5)

                nc.sync.dma_start(
                    out=r_v[:, c * CB : (c + 1) * CB, :].rearrange(
                        "p (q d) m -> p q d m", q=QUADS
                    ),
                    in_=r_t,
                )

    nc.compile()
    return nc


def _get_program(n_local=N_LOCAL):
    key = (n_local, MM_DTYPE, USE_DIVIDE)
    if key not in _CACHE:
        _CACHE[key] = _build_program(n_local)
    return _CACHE[key]


def kernel(theta, basis_mu, basis_sigma):
    from concourse.bass_utils import run_bass_kernel_spmd

    theta = np.ascontiguousarray(theta, dtype=np.float32)
    btab = _build_b_table(np.asarray(basis_mu), np.asarray(basis_sigma))

    nc = _get_program()
    in_maps = [
        {
            "theta": theta[i * N_LOCAL : (i + 1) * N_LOCAL],
            "btab": btab,
        }
        for i in range(N_CORES)
    ]
    res = run_bass_kernel_spmd(nc, in_maps, core_ids=list(range(N_CORES)))
    return np.concatenate([r["r"] for r in res.results], axis=0)

